# revision 3
# baseline (speedup 1.0000x reference)
"""Chamfer distance kernel for Trainium2 (8 NeuronCores, batch-parallel) — v2.

B=8 batches, one per core (SPMD). Per core (N=M=8192, 3-D points), two
direction passes (rows = x1 points, then rows = x2 points); 64 row-blocks
of 128 points each; per block, 4 column-groups of 2048.

Key ideas vs the K=5 fp32 baseline (3.4 ms of mostly fp32-matmul + DVE):

1. bf16 triple-split matmul (K=24): each coordinate/norm row is split into
   hi/mid/lo bf16 parts; the 6 product pairs that matter (hh, hm, mh, hl,
   lh, mm) plus the norm rows are laid out as 24 stationary/moving row
   pairs, so ONE bf16 matmul per 512-column chunk produces d to ~1e-7
   absolute accuracy (1 argmin flip in 65536 rows on the reference data) at
   ~4.5x the fp32 matmul rate.

2. Fused suffix-min scan: groups are processed g=3..0. One DVE
   tensor_scalar(op0=min vs previous suffix-min, op1=min reduce,
   accum_out) per group reads PSUM once and yields
   s_g = min(group_g min, s_{g+1}) with zero extra ops. s_0 is the row
   (block) min -> dist output.

3. Exact-indicator strips: ACT writes t = Exp((s_g - d)*2^67) per group
   into a bf16 strip. 2^67 is a power of two so s_g*2^67 is exact: the
   strip is exactly 1.0 where d attains the suffix min and 0.0 elsewhere
   (any nonzero f32 gap scales to < -128 before Exp). Groups biased by
   the SUFFIX min mean: groups before the winning group have no 1.0 at
   all, so the FIRST 1.0 in the 8192-wide strip is the global argmin,
   with np.argmin's first-occurrence tie semantics preserved exactly.

4. ONE bf16 max_index per block (query = constant 1.0) extracts the
   argmin; max_index on bf16 runs ~4x faster than on fp32.

Host does the bf16 splitting (prep) and the final relu/slicing.
"""

import numpy as np
import ml_dtypes

import concourse.bacc as bacc
import concourse.mybir as mybir
from concourse import tile
from concourse.bass_utils import run_bass_kernel_spmd

F32 = mybir.dt.float32
BF16 = mybir.dt.bfloat16
U32 = mybir.dt.uint32
AF = mybir.ActivationFunctionType
ALU = mybir.AluOpType

BF = ml_dtypes.bfloat16
SCALE = float(2.0 ** 67)

_PROGRAM_CACHE = {}


def _build_program(n_pts=8192, n_cores=8, repeat=1):
    key = (n_pts, n_cores, repeat)
    if key in _PROGRAM_CACHE:
        return _PROGRAM_CACHE[key]

    NB = n_pts // 128          # row blocks per direction
    NG = 4                     # column groups per block
    GW = n_pts // NG           # group width (2048)

    nc = bacc.Bacc("TRN2", target_bir_lowering=False, debug=False,
                   num_devices=n_cores)
    uu = nc.dram_tensor("uu", [96, n_pts], BF16, kind="ExternalInput")
    s1o = nc.dram_tensor("s1", [128, NB * NG], F32, kind="ExternalOutput")
    x1o = nc.dram_tensor("x1", [128, NB * 16], U32, kind="ExternalOutput")
    s2o = nc.dram_tensor("s2", [128, NB * NG], F32, kind="ExternalOutput")
    x2o = nc.dram_tensor("x2", [128, NB * 16], U32, kind="ExternalOutput")

    with tile.TileContext(nc) as tc:
        with tc.tile_pool(name="persist", bufs=1) as persist:
            # rows 0-23: A-form (stationary side), rows 32-55: B-form
            # (moving side). Pass A pairs U1[0:24] x U2[0:24]; pass B pairs
            # U2[32:56] x U1[32:56] (matmul operands share a base quadrant).
            U1 = persist.tile([56, n_pts], BF16, tag="U1")
            U2 = persist.tile([56, n_pts], BF16, tag="U2")
            q1 = persist.tile([128, 8], BF16, tag="q1")
            big = persist.tile([128, 1], F32, tag="big")
            sacc1 = persist.tile([128, NB * NG], F32, tag="sacc1")
            sacc2 = persist.tile([128, NB * NG], F32, tag="sacc2")
            xacc1 = persist.tile([128, NB * 16], U32, tag="xacc1")
            xacc2 = persist.tile([128, NB * 16], U32, tag="xacc2")

            nc.vector.memset(q1[:], 1.0)
            nc.vector.memset(big[:], 3.4e38)
            nc.sync.dma_start(U1[0:24, :], uu.ap()[0:24, :])
            nc.sync.dma_start(U1[32:56, :], uu.ap()[24:48, :])
            nc.sync.dma_start(U2[0:24, :], uu.ap()[72:96, :])
            nc.sync.dma_start(U2[32:56, :], uu.ap()[48:72, :])

            dirs = ((U1, U2, 0, sacc1, xacc1),
                    (U2, U1, 32, sacc2, xacc2))

            with tc.tile_pool(name="psum", bufs=2, space="PSUM") as pspool, \
                 tc.tile_pool(name="tb", bufs=4) as tbpool:
                for _ in range(repeat):
                    # The two direction passes are interleaved block by block
                    # so each pass's DVE work hides the other's ACT/PSUM
                    # dependency chain; each block's max_index is emitted one
                    # interleave step late for the same reason.
                    pend = [None, None]

                    def emit_block(di, nb):
                        lhsU, rhsU, base, sacc, xacc = dirs[di]
                        tb = tbpool.tile([128, n_pts], BF16, tag="tb")
                        lhs = lhsU[base:base + 24, nb * 128:(nb + 1) * 128]
                        for g in range(NG - 1, -1, -1):
                            ps = pspool.tile([128, GW], F32, tag="ps")
                            for q in range(GW // 512):
                                c0 = g * GW + q * 512
                                nc.tensor.matmul(
                                    ps[:, q * 512:(q + 1) * 512],
                                    lhs,
                                    rhsU[base:base + 24, c0:c0 + 512],
                                    start=True, stop=True)
                            c = nb * NG + g
                            sprev = (big[:, 0:1] if g == NG - 1
                                     else sacc[:, c + 1:c + 2])
                            # suffix min: s_g = min(min(group), s_{g+1}); the
                            # elementwise out is junk (overwritten by the Exp)
                            nc.vector.tensor_scalar(
                                out=tb[:, g * GW:(g + 1) * GW],
                                in0=ps[:], scalar1=sprev, scalar2=None,
                                op0=ALU.min, op1=ALU.min,
                                accum_out=sacc[:, c:c + 1])
                            # psum holds d*2^67 (A-rows pre-scaled on the
                            # host), so the suffix min IS the Exp bias
                            nc.scalar.activation(
                                tb[:, g * GW:(g + 1) * GW], ps[:],
                                AF.Exp, bias=sacc[:, c:c + 1], scale=-1.0)
                            if g == 2:
                                # upper half (groups 3,2) is final once its
                                # exps land; scan it while the lower half's
                                # matmul/ts chain is still draining
                                nc.vector.max_index(
                                    xacc[:, nb * 16 + 8:nb * 16 + 16],
                                    q1[:], tb[:, 2 * GW:4 * GW])
                        pend[di] = (tb, nb)

                    def emit_maxidx(di):
                        if pend[di] is None:
                            return
                        xacc = dirs[di][4]
                        ptb, pnb = pend[di]
                        nc.vector.max_index(
                            xacc[:, pnb * 16:pnb * 16 + 8], q1[:],
                            ptb[:, 0:2 * GW])
                        pend[di] = None

                    for nb in range(NB):
                        emit_block(0, nb)
                        emit_maxidx(1)
                        emit_block(1, nb)
                        emit_maxidx(0)
                    emit_maxidx(1)

            nc.sync.dma_start(s1o.ap(), sacc1[:])
            nc.sync.dma_start(x1o.ap(), xacc1[:])
            nc.sync.dma_start(s2o.ap(), sacc2[:])
            nc.sync.dma_start(x2o.ap(), xacc2[:])

    nc.compile()
    _PROGRAM_CACHE[key] = nc
    return nc


def _split3(v):
    """bf16 triple split: v ~= h + m + l with each part bf16-representable
    (returned as f32 numpy arrays)."""
    h = v.astype(BF).astype(np.float32)
    r = (v - h).astype(np.float32)
    m = r.astype(BF).astype(np.float32)
    l = (r - m).astype(BF).astype(np.float32)
    return h, m, l


def _forms(xyz):
    """[N,3] f32 -> (A, B) [24, N] bf16 homogeneous triple-split forms.

    Row pairing (A row i) * (B row i), summed by the PE:
      0-2  (-2x_h, x'_h)   3-5  (-2x_h, x'_m)   6-8  (-2x_m, x'_h)
      9-11 (-2x_h, x'_l)  12-14 (-2x_l, x'_h)  15-17 (-2x_m, x'_m)
      18-20 (1, n'_{h,m,l})  21-23 (n_{h,m,l}, 1)
    = n + n' - 2(hh+hm+mh+hl+lh+mm) ~= squared distance.
    """
    x = np.ascontiguousarray(xyz.T).astype(np.float32)      # [3, N]
    n = (x * x).sum(0, dtype=np.float32)[None, :]           # [1, N]
    s = (-2.0 * x).astype(np.float32)
    sh, sm, sl = _split3(s)
    xh, xm, xl = _split3(x)
    nh, nm, nl = _split3(n)
    ones = np.ones_like(n)
    A = (np.concatenate([sh, sh, sm, sh, sl, sm, ones, ones, ones,
                         nh, nm, nl]) * SCALE).astype(BF)
    Bf = np.concatenate([xh, xm, xh, xl, xh, xm, nh, nm, nl,
                         ones, ones, ones]).astype(BF)
    return A, Bf


def kernel(xyz1: np.ndarray, xyz2: np.ndarray, repeat: int = 1):
    xyz1 = np.asarray(xyz1, dtype=np.float32)
    xyz2 = np.asarray(xyz2, dtype=np.float32)
    B, N, _ = xyz1.shape
    M = xyz2.shape[1]
    assert B == 8 and N == 8192 and M == 8192, (B, N, M)

    nc = _build_program(N, B, repeat)

    in_maps = []
    for b in range(B):
        A1, B1 = _forms(xyz1[b])
        A2, B2 = _forms(xyz2[b])
        in_maps.append({"uu": np.concatenate([A1, B1, A2, B2])})
    res = run_bass_kernel_spmd(nc, in_maps, list(range(B)))

    NB = N // 128
    dist1 = np.empty((B, N), np.float32)
    dist2 = np.empty((B, M), np.float32)
    idx1 = np.empty((B, N), np.int32)
    idx2 = np.empty((B, M), np.int32)
    for b in range(B):
        r = res.results[b]
        for s_name, x_name, dist, idx in (("s1", "x1", dist1, idx1),
                                          ("s2", "x2", dist2, idx2)):
            s = np.asarray(r[s_name])          # [128, NB*4]
            xi = np.asarray(r[x_name])         # [128, NB*16] u32
            vmin = s[:, 0::4] * (1.0 / SCALE)  # s_0 per block -> [128, NB]
            ixa = xi[:, 0::16]                 # lower-half scan, slot 0
            ixb = xi[:, 8::16]                 # upper-half scan, slot 0
            ix = np.where(ixa != np.uint32(0xFFFFFFFF),
                          ixa, ixb + 4096).astype(np.int64)
            dist[b] = np.maximum(vmin, 0.0).T.reshape(-1)
            idx[b] = ix.T.reshape(-1).astype(np.int32)
    return dist1, dist2, idx1, idx2


# revision 4
# speedup vs baseline: 1.0776x; 1.0776x over previous
"""Chamfer distance kernel for Trainium2 — v3 (bf16-domain min + host repair).

v2 hit the DVE 1x floor: an f32 min-scan from PSUM (2.26us/2048) plus a
bf16 position-scan (1x) per block. v3 moves the min-scan into the bf16
domain where tensor_scalar runs at 2x:

- ACT copies each PSUM group to a bf16 strip (same cost as v2's Exp).
- bf16 rounding is monotone, so min(bf16(d)) == bf16(min(d)) EXACTLY; the
  chained DVE tensor_scalar(min, accum) now reads bf16 SBUF at 2x.
- max_index queries the bf16 block-min over the raw bf16 strips: the first
  position whose bf16 value equals the bf16 min. If that bucket holds only
  ONE element, this is exactly np.argmin of the f32 matrix.
- Repeated query slots return successive occurrences (slot1 = second match,
  0xFFFFFFFF if none), so multi-match rows (~1% - where bf16 buckets
  collide at the min) are detected for FREE; the host recomputes those few
  rows exactly in f32 numpy (exact argmin AND exact dist).
- dist for clean rows = bf16(min d), rel err ~1e-3 << the 2e-2 gate.
"""

import numpy as np
import ml_dtypes

import concourse.bacc as bacc
import concourse.mybir as mybir
from concourse import tile
from concourse.bass_utils import run_bass_kernel_spmd

F32 = mybir.dt.float32
BF16 = mybir.dt.bfloat16
U32 = mybir.dt.uint32
AF = mybir.ActivationFunctionType
ALU = mybir.AluOpType

BF = ml_dtypes.bfloat16
MISS = np.uint32(0xFFFFFFFF)

_PROGRAM_CACHE = {}


def _build_program(n_pts=8192, n_cores=8, repeat=1):
    key = (n_pts, n_cores, repeat)
    if key in _PROGRAM_CACHE:
        return _PROGRAM_CACHE[key]

    NB = n_pts // 128
    NG = 4
    GW = n_pts // NG

    nc = bacc.Bacc("TRN2", target_bir_lowering=False, debug=False,
                   num_devices=n_cores)
    uu = nc.dram_tensor("uu", [96, n_pts], BF16, kind="ExternalInput")
    s1o = nc.dram_tensor("s1", [128, NB * NG], F32, kind="ExternalOutput")
    x1o = nc.dram_tensor("x1", [128, NB * 16], U32, kind="ExternalOutput")
    s2o = nc.dram_tensor("s2", [128, NB * NG], F32, kind="ExternalOutput")
    x2o = nc.dram_tensor("x2", [128, NB * 16], U32, kind="ExternalOutput")

    with tile.TileContext(nc) as tc:
        with tc.tile_pool(name="persist", bufs=1) as persist:
            U1 = persist.tile([56, n_pts], BF16, tag="U1")
            U2 = persist.tile([56, n_pts], BF16, tag="U2")
            big = persist.tile([128, 1], F32, tag="big")
            sacc1 = persist.tile([128, NB * NG], F32, tag="sacc1")
            sacc2 = persist.tile([128, NB * NG], F32, tag="sacc2")
            xacc1 = persist.tile([128, NB * 16], U32, tag="xacc1")
            xacc2 = persist.tile([128, NB * 16], U32, tag="xacc2")

            nc.vector.memset(big[:], 3.4e38)
            nc.sync.dma_start(U1[0:24, :], uu.ap()[0:24, :])
            nc.sync.dma_start(U1[32:56, :], uu.ap()[24:48, :])
            nc.sync.dma_start(U2[0:24, :], uu.ap()[72:96, :])
            nc.sync.dma_start(U2[32:56, :], uu.ap()[48:72, :])

            dirs = ((U1, U2, 0, sacc1, xacc1),
                    (U2, U1, 32, sacc2, xacc2))

            with tc.tile_pool(name="psum", bufs=2, space="PSUM") as pspool, \
                 tc.tile_pool(name="tb", bufs=4) as tbpool, \
                 tc.tile_pool(name="jk", bufs=2) as jkpool, \
                 tc.tile_pool(name="q8", bufs=4) as q8pool:
                for _ in range(repeat):
                    pend = [None, None]

                    def emit_block(di, nb):
                        lhsU, rhsU, base, sacc, xacc = dirs[di]
                        tb = tbpool.tile([128, n_pts], BF16, tag="tb")
                        lhs = lhsU[base:base + 24, nb * 128:(nb + 1) * 128]
                        for g in range(NG - 1, -1, -1):
                            ps = pspool.tile([128, GW], F32, tag="ps")
                            for q in range(GW // 512):
                                c0 = g * GW + q * 512
                                nc.tensor.matmul(
                                    ps[:, q * 512:(q + 1) * 512],
                                    lhs,
                                    rhsU[base:base + 24, c0:c0 + 512],
                                    start=True, stop=True)
                            c = nb * NG + g
                            # bf16 value strip (monotone cast) frees PSUM
                            nc.scalar.activation(
                                tb[:, g * GW:(g + 1) * GW], ps[:], AF.Copy)
                            sprev = (big[:, 0:1] if g == NG - 1
                                     else sacc[:, c + 1:c + 2])
                            # chained block-min on bf16 SBUF (fast mode);
                            # elementwise out goes to a junk tile so the
                            # value strip stays intact for max_index
                            jk = jkpool.tile([128, GW], BF16, tag="jk")
                            nc.vector.tensor_scalar(
                                out=jk[:],
                                in0=tb[:, g * GW:(g + 1) * GW],
                                scalar1=sprev, scalar2=None,
                                op0=ALU.min, op1=ALU.min,
                                accum_out=sacc[:, c:c + 1])
                        # bf16 query = block min (exactly representable)
                        q8t = q8pool.tile([128, 8], BF16, tag="q8t")
                        nc.scalar.activation(
                            q8t[:],
                            sacc[:, nb * NG:nb * NG + 1].broadcast_to((128, 8)),
                            AF.Copy)
                        pend[di] = (tb, q8t, nb)

                    def emit_maxidx(di):
                        if pend[di] is None:
                            return
                        xacc = dirs[di][4]
                        ptb, pq, pnb = pend[di]
                        nc.vector.max_index(
                            xacc[:, pnb * 16:pnb * 16 + 8], pq[:],
                            ptb[:, 0:2 * GW])
                        nc.vector.max_index(
                            xacc[:, pnb * 16 + 8:pnb * 16 + 16], pq[:],
                            ptb[:, 2 * GW:4 * GW])
                        pend[di] = None

                    for nb in range(NB):
                        emit_block(0, nb)
                        emit_maxidx(1)
                        emit_block(1, nb)
                        emit_maxidx(0)
                    emit_maxidx(1)

            nc.sync.dma_start(s1o.ap(), sacc1[:])
            nc.sync.dma_start(x1o.ap(), xacc1[:])
            nc.sync.dma_start(s2o.ap(), sacc2[:])
            nc.sync.dma_start(x2o.ap(), xacc2[:])

    nc.compile()
    _PROGRAM_CACHE[key] = nc
    return nc


def _split3(v):
    h = v.astype(BF).astype(np.float32)
    r = (v - h).astype(np.float32)
    m = r.astype(BF).astype(np.float32)
    l = (r - m).astype(BF).astype(np.float32)
    return h, m, l


def _forms(xyz):
    """[N,3] f32 -> (A, B) [24, N] bf16 triple-split homogeneous forms."""
    x = np.ascontiguousarray(xyz.T).astype(np.float32)
    n = (x * x).sum(0, dtype=np.float32)[None, :]
    s = (-2.0 * x).astype(np.float32)
    sh, sm, sl = _split3(s)
    xh, xm, xl = _split3(x)
    nh, nm, nl = _split3(n)
    ones = np.ones_like(n)
    A = np.concatenate([sh, sh, sm, sh, sl, sm, ones, ones, ones,
                        nh, nm, nl]).astype(BF)
    Bf = np.concatenate([xh, xm, xh, xl, xh, xm, nh, nm, nl,
                         ones, ones, ones]).astype(BF)
    return A, Bf


def kernel(xyz1: np.ndarray, xyz2: np.ndarray, repeat: int = 1):
    xyz1 = np.asarray(xyz1, dtype=np.float32)
    xyz2 = np.asarray(xyz2, dtype=np.float32)
    B, N, _ = xyz1.shape
    M = xyz2.shape[1]
    assert B == 8 and N == 8192 and M == 8192, (B, N, M)

    nc = _build_program(N, B, repeat)

    in_maps = []
    for b in range(B):
        A1, B1 = _forms(xyz1[b])
        A2, B2 = _forms(xyz2[b])
        in_maps.append({"uu": np.concatenate([A1, B1, A2, B2])})
    res = run_bass_kernel_spmd(nc, in_maps, list(range(B)))

    NB = N // 128
    dist1 = np.empty((B, N), np.float32)
    dist2 = np.empty((B, M), np.float32)
    idx1 = np.empty((B, N), np.int32)
    idx2 = np.empty((B, M), np.int32)
    for b in range(B):
        r = res.results[b]
        for dirn, (s_name, x_name, dist, idx, Xq, Xc) in enumerate((
                ("s1", "x1", dist1, idx1, xyz1[b], xyz2[b]),
                ("s2", "x2", dist2, idx2, xyz2[b], xyz1[b]))):
            s = np.asarray(r[s_name])
            xi = np.asarray(r[x_name])
            vmin = s[:, 0::4]                   # [128, NB] block min (bf16)
            lo0, lo1 = xi[:, 0::16], xi[:, 1::16]
            hi0, hi1 = xi[:, 8::16], xi[:, 9::16]
            lo_hit = lo0 != MISS
            ix = np.where(lo_hit, lo0, hi0 + 4096).astype(np.int64)
            # multi-match rows: second occurrence anywhere
            multi = np.where(lo_hit, (lo1 != MISS) | (hi0 != MISS),
                             hi1 != MISS)
            d_full = np.maximum(vmin, 0.0).T.reshape(-1)
            i_full = ix.T.reshape(-1)
            flag = multi.T.reshape(-1)
            rows = np.nonzero(flag)[0]
            if rows.size:
                # exact f32 repair for bf16-collision rows
                q = Xq[rows]                                    # [R, 3]
                d = (q * q).sum(1)[:, None] + (Xc * Xc).sum(1)[None, :] \
                    - 2.0 * (q @ Xc.T)
                d = np.maximum(d.astype(np.float32), 0.0)
                i_full[rows] = d.argmin(1)
                d_full[rows] = d.min(1)
            dist[b] = d_full
            idx[b] = i_full.astype(np.int32)
    return dist1, dist2, idx1, idx2
